# revision 9
# baseline (speedup 1.0000x reference)
"""HDNet 3-layer DAG-of-matmuls kernel for 8 TRN2 NeuronCores.

Math: out = concat(x0,x1) @ W0 @ W1 @ W2  (the concat/split DAG routing is
identity on the channel axis), with B=4096, C=1024, so X=[4096,2048] and
each W=[2048,2048].

Sharding: data-parallel over batch. Each core gets 512 rows of X and full
(replicated) weights. Per core:
  - A0 = X_c^T  (channels on partitions, batch on free), via XBAR DMA
    transpose on the Scalar HWDGE queue (PE does no transpose work)
  - layers 1,2: Z^T[m,:] = sum_k W[k,m].T @ A[k,:]   (activations stay
    transposed, weights are the stationary operand)
  - layer 3 swaps operands: Z3[mb,n] = sum_k A[k,mb].T @ W2[k,n], which
    yields the output in natural [batch, channel] layout directly.
All matmuls in bf16 with fp32 PSUM accumulation. Weights and X are cast
fp32->bf16 inside the (software-DGE) DMA itself.

Loop order is k-OUTER with 8 concurrent PSUM accumulation chains per group
so the in-order PE consumes weight strips as they stream from HBM. Weight
strips are loaded as column halves (A: cols 0..1023, B: cols 1024..2047);
group 0 only needs the A halves, halving the ramp-up stall.
"""

import numpy as np

import concourse.bass as bass
import concourse.bacc as bacc
import concourse.mybir as mybir
import concourse.tile as tile
from concourse.bass_utils import run_bass_kernel_spmd

B = 4096
C = 1024
NCORES = 8
BC = B // NCORES          # 512 rows per core
K = 2 * C                 # 2048 contraction / channel dim
P = 128                   # partitions
KT = K // P               # 16 k-tiles
MT = K // P               # 16 m-tiles (output channel tiles of 128)
NBAT = BC                 # 512, batch free size (fits one PSUM bank in f32)
G = 8                     # concurrent PSUM chains per group
KH = K // 2               # 1024, weight column half

F32 = mybir.dt.float32
BF16 = mybir.dt.bfloat16

_built = None


def _build():
    nc = bacc.Bacc(num_swdge_queues=4)
    x0 = nc.declare_dram_parameter("x0", [BC, C], F32, isOutput=False)
    x1 = nc.declare_dram_parameter("x1", [BC, C], F32, isOutput=False)
    ws = [
        nc.declare_dram_parameter(f"W{i}", [K, K], F32, isOutput=False)
        for i in range(3)
    ]
    out = nc.declare_dram_parameter("out", [BC, K], F32, isOutput=True)

    with tile.TileContext(nc) as tc:
        with (
            tc.tile_pool(name="xpool", bufs=8) as xpool,      # bf16 X row-tiles
            tc.tile_pool(name="wpool", bufs=2) as wpool,      # bf16 weight half-strips
            tc.tile_pool(name="act", bufs=2) as act,          # bf16 activation strips
            tc.tile_pool(name="outp", bufs=4) as outp,        # f32 out staging
            tc.tile_pool(name="psum", bufs=G, space=bass.MemorySpace.PSUM) as psum,
        ):
            # ---- X: casting DMA to bf16, then XBAR DMA-transpose into A0 ----
            a_in = []
            for k in range(KT):
                a_in.append(act.tile([P, NBAT], BF16, tag=f"a{k}", name=f"a_in{k}"))

            for r in range(BC // P):  # 4 row-tiles of X
                xh = []
                for h, src in enumerate((x0, x1)):
                    xb = xpool.tile([P, C], BF16, tag=f"xb{h}", name=f"xb{r}_{h}")
                    nc.gpsimd.dma_start(xb[:], src[r * P:(r + 1) * P, :])
                    xh.append(xb)
                for k in range(KT):
                    src = xh[k // (KT // 2)]
                    kk = k % (KT // 2)
                    nc.scalar.dma_start(
                        a_in[k][:, r * P:(r + 1) * P],
                        src[:, kk * P:(kk + 1) * P],
                        transpose=True,
                    )

            # ---- weight loader: two column halves per k strip ----
            def load_w(layer):
                ha, hb = [], []
                for half, lst in ((0, ha), (1, hb)):
                    for k in range(KT):
                        wb = wpool.tile(
                            [P, KH], BF16,
                            tag=f"w{half}_{k}", name=f"w{layer}_{half}_{k}",
                        )
                        nc.gpsimd.dma_start(
                            wb[:],
                            ws[layer][k * P:(k + 1) * P, half * KH:(half + 1) * KH],
                        )
                        lst.append(wb)
                return ha, hb

            # ---- transposed-activation layer: k-outer, G chains per group ----
            def layer_t(whalves, ain, lname):
                nxt = []
                for gi, half in enumerate(whalves):  # group 0: cols 0..1023
                    chains = [
                        psum.tile([P, NBAT], F32, tag="ps",
                                  name=f"ps{lname}_{gi * G + j}")
                        for j in range(G)
                    ]
                    for k in range(KT):
                        for j in range(G):
                            nc.tensor.matmul(
                                chains[j][:],
                                half[k][:, j * P:(j + 1) * P],
                                ain[k][:],
                                start=(k == 0),
                                stop=(k == KT - 1),
                            )
                    for j in range(G):
                        m = gi * G + j
                        ao = act.tile(
                            [P, NBAT], BF16, tag=f"a{m}", name=f"a{lname}_{m}"
                        )
                        nc.vector.tensor_copy(ao[:], chains[j][:])
                        nxt.append(ao)
                return nxt

            w0a, w0b = load_w(0)
            a1 = layer_t((w0a, w0b), a_in, "L1")
            w1a, w1b = load_w(1)     # emitted after L1 so W0 wins DMA priority
            a2 = layer_t((w1a, w1b), a1, "L2")
            w2a, w2b = load_w(2)

            # ---- layer 3: natural-layout output, k-outer over (mb, n) pairs ----
            # n-major grouping: group 0 touches only the A halves of W2.
            pairs = [(mb, n) for n in range(K // NBAT) for mb in range(BC // P)]
            for g0 in range(0, len(pairs), G):
                grp = pairs[g0:g0 + G]
                chains = [
                    psum.tile([P, NBAT], F32, tag="ps", name=f"psL3_{g0 + j}")
                    for j in range(len(grp))
                ]
                for k in range(KT):
                    for j, (mb, n) in enumerate(grp):
                        half = (w2a, w2b)[n // 2]
                        nc.tensor.matmul(
                            chains[j][:],
                            a2[k][:, mb * P:(mb + 1) * P],
                            half[k][:, (n % 2) * NBAT:(n % 2 + 1) * NBAT],
                            start=(k == 0),
                            stop=(k == KT - 1),
                        )
                for j, (mb, n) in enumerate(grp):
                    ob = outp.tile([P, NBAT], F32, tag="ob", name=f"ob{g0 + j}")
                    nc.vector.tensor_copy(ob[:], chains[j][:])
                    nc.sync.dma_start(
                        out[mb * P:(mb + 1) * P, n * NBAT:(n + 1) * NBAT], ob[:]
                    )

    nc.finalize()
    return nc


def _run(inputs, trace=False, **kw):
    global _built
    if _built is None:
        _built = _build()
    nc = _built
    in_maps = []
    for c in range(NCORES):
        sl = slice(c * BC, (c + 1) * BC)
        in_maps.append({
            "x0": np.ascontiguousarray(inputs["x0"][sl]),
            "x1": np.ascontiguousarray(inputs["x1"][sl]),
            "W0": inputs["W0"],
            "W1": inputs["W1"],
            "W2": inputs["W2"],
        })
    res = run_bass_kernel_spmd(nc, in_maps, list(range(NCORES)), trace=trace, **kw)
    out = np.concatenate([res.results[c]["out"] for c in range(NCORES)], axis=0)
    return out, res


def kernel(**inputs):
    out, _ = _run(inputs)
    return out


# revision 11
# speedup vs baseline: 1.5715x; 1.5715x over previous
"""HDNet 3-layer DAG-of-matmuls kernel for 8 TRN2 NeuronCores.

Math: out = concat(x0,x1) @ W0 @ W1 @ W2  (the concat/split DAG routing is
identity on the channel axis), with B=4096, C=1024, so X=[4096,2048] and
each W=[2048,2048].

Sharding: data-parallel over batch. Each core gets 512 rows of X and full
(replicated) weights. Per core:
  - A0 = X_c^T  (channels on partitions, batch on free), via XBAR DMA
    transpose on the Scalar HWDGE queue (PE does no transpose work)
  - layers 1,2: Z^T[m,:] = sum_k W[k,m].T @ A[k,:]   (activations stay
    transposed, weights are the stationary operand)
  - layer 3 swaps operands: Z3[mb,n] = sum_k A[k,mb].T @ W2[k,n], which
    yields the output in natural [batch, channel] layout directly.
All matmuls in bf16 with fp32 PSUM accumulation. Weights and X are cast
fp32->bf16 inside the (software-DGE) DMA itself.

Loop order is k-OUTER with 8 concurrent PSUM accumulation chains per group
so the in-order PE consumes weight strips as they stream from HBM. Weight
strips are loaded as column halves (A: cols 0..1023, B: cols 1024..2047);
group 0 only needs the A halves, halving the ramp-up stall.
"""

import numpy as np

import concourse.bass as bass
import concourse.bacc as bacc
import concourse.mybir as mybir
import concourse.tile as tile
from concourse.bass_utils import run_bass_kernel_spmd
from concourse.masks import make_identity

B = 4096
C = 1024
NCORES = 8
BC = B // NCORES          # 512 rows per core
K = 2 * C                 # 2048 contraction / channel dim
P = 128                   # partitions
KT = K // P               # 16 k-tiles
MT = K // P               # 16 m-tiles (output channel tiles of 128)
NBAT = BC                 # 512, batch free size (fits one PSUM bank in f32)
G = 8                     # concurrent PSUM chains per group
KH = K // 2               # 1024, weight column half

F32 = mybir.dt.float32
BF16 = mybir.dt.bfloat16

_built = None


def _build():
    nc = bacc.Bacc(num_swdge_queues=4)
    x0 = nc.declare_dram_parameter("x0", [BC, C], F32, isOutput=False)
    x1 = nc.declare_dram_parameter("x1", [BC, C], F32, isOutput=False)
    ws = [
        nc.declare_dram_parameter(f"W{i}", [K, K], F32, isOutput=False)
        for i in range(3)
    ]
    out = nc.declare_dram_parameter("out", [BC, K], F32, isOutput=True)

    with tile.TileContext(nc) as tc:
        with (
            tc.tile_pool(name="xpool", bufs=8) as xpool,      # bf16 X row-tiles
            tc.tile_pool(name="wpool", bufs=2) as wpool,      # bf16 weight half-strips
            tc.tile_pool(name="act", bufs=2) as act,          # bf16 activation strips
            tc.tile_pool(name="outp", bufs=4) as outp,        # f32 out staging
            tc.tile_pool(name="psum", bufs=G, space=bass.MemorySpace.PSUM) as psum,
        ):
            # ---- X: casting DMA to bf16, then PE transpose into A0 ----
            ident = xpool.tile([P, P], BF16, tag="ident", name="ident")
            make_identity(nc, ident[:])

            a_in = []
            for k in range(KT):
                a_in.append(act.tile([P, NBAT], BF16, tag=f"a{k}", name=f"a_in{k}"))

            for r in range(BC // P):  # 4 row-tiles of X
                xh = []
                for h, src in enumerate((x0, x1)):
                    xb = xpool.tile([P, C], BF16, tag=f"xb{h}", name=f"xb{r}_{h}")
                    nc.gpsimd.dma_start(xb[:], src[r * P:(r + 1) * P, :])
                    xh.append(xb)
                for k in range(KT):
                    src = xh[k // (KT // 2)]
                    kk = k % (KT // 2)
                    pt = psum.tile([P, P], BF16, tag="ps", name=f"pt{r}_{k}")
                    nc.tensor.transpose(pt[:], src[:, kk * P:(kk + 1) * P], ident[:])
                    nc.vector.tensor_copy(a_in[k][:, r * P:(r + 1) * P], pt[:])

            # ---- weight loader: two column halves per k strip ----
            def load_w(layer):
                ha, hb = [], []
                for half, lst in ((0, ha), (1, hb)):
                    for k in range(KT):
                        wb = wpool.tile(
                            [P, KH], BF16,
                            tag=f"w{half}_{k}", name=f"w{layer}_{half}_{k}",
                        )
                        nc.gpsimd.dma_start(
                            wb[:],
                            ws[layer][k * P:(k + 1) * P, half * KH:(half + 1) * KH],
                        )
                        lst.append(wb)
                return ha, hb

            # ---- transposed-activation layer: k-outer, G chains per group ----
            def layer_t(whalves, ain, lname):
                nxt = []
                for gi, half in enumerate(whalves):  # group 0: cols 0..1023
                    chains = [
                        psum.tile([P, NBAT], F32, tag="ps",
                                  name=f"ps{lname}_{gi * G + j}")
                        for j in range(G)
                    ]
                    for k in range(KT):
                        for j in range(G):
                            nc.tensor.matmul(
                                chains[j][:],
                                half[k][:, j * P:(j + 1) * P],
                                ain[k][:],
                                start=(k == 0),
                                stop=(k == KT - 1),
                            )
                    for j in range(G):
                        m = gi * G + j
                        ao = act.tile(
                            [P, NBAT], BF16, tag=f"a{m}", name=f"a{lname}_{m}"
                        )
                        nc.vector.tensor_copy(ao[:], chains[j][:])
                        nxt.append(ao)
                return nxt

            w0a, w0b = load_w(0)
            a1 = layer_t((w0a, w0b), a_in, "L1")
            w1a, w1b = load_w(1)     # emitted after L1 so W0 wins DMA priority
            a2 = layer_t((w1a, w1b), a1, "L2")
            w2a, w2b = load_w(2)

            # ---- layer 3: natural-layout output, k-outer over (mb, n) pairs ----
            # n-major grouping: group 0 touches only the A halves of W2.
            pairs = [(mb, n) for n in range(K // NBAT) for mb in range(BC // P)]
            for g0 in range(0, len(pairs), G):
                grp = pairs[g0:g0 + G]
                chains = [
                    psum.tile([P, NBAT], F32, tag="ps", name=f"psL3_{g0 + j}")
                    for j in range(len(grp))
                ]
                for k in range(KT):
                    for j, (mb, n) in enumerate(grp):
                        half = (w2a, w2b)[n // 2]
                        nc.tensor.matmul(
                            chains[j][:],
                            a2[k][:, mb * P:(mb + 1) * P],
                            half[k][:, (n % 2) * NBAT:(n % 2 + 1) * NBAT],
                            start=(k == 0),
                            stop=(k == KT - 1),
                        )
                for j, (mb, n) in enumerate(grp):
                    ob = outp.tile([P, NBAT], F32, tag="ob", name=f"ob{g0 + j}")
                    nc.vector.tensor_copy(ob[:], chains[j][:])
                    nc.sync.dma_start(
                        out[mb * P:(mb + 1) * P, n * NBAT:(n + 1) * NBAT], ob[:]
                    )

    nc.finalize()
    return nc


def _run(inputs, trace=False, **kw):
    global _built
    if _built is None:
        _built = _build()
    nc = _built
    in_maps = []
    for c in range(NCORES):
        sl = slice(c * BC, (c + 1) * BC)
        in_maps.append({
            "x0": np.ascontiguousarray(inputs["x0"][sl]),
            "x1": np.ascontiguousarray(inputs["x1"][sl]),
            "W0": inputs["W0"],
            "W1": inputs["W1"],
            "W2": inputs["W2"],
        })
    res = run_bass_kernel_spmd(nc, in_maps, list(range(NCORES)), trace=trace, **kw)
    out = np.concatenate([res.results[c]["out"] for c in range(NCORES)], axis=0)
    return out, res


def kernel(**inputs):
    out, _ = _run(inputs)
    return out


# revision 12
# speedup vs baseline: 1.6173x; 1.0292x over previous
"""HDNet 3-layer DAG-of-matmuls kernel for 8 TRN2 NeuronCores.

Math: out = concat(x0,x1) @ W0 @ W1 @ W2  (the concat/split DAG routing is
identity on the channel axis), with B=4096, C=1024, so X=[4096,2048] and
each W=[2048,2048].

Sharding: data-parallel over batch. Each core gets 512 rows of X and full
(replicated) weights. Per core:
  - A0 = X_c^T  (channels on partitions, batch on free), via XBAR DMA
    transpose on the Scalar HWDGE queue (PE does no transpose work)
  - layers 1,2: Z^T[m,:] = sum_k W[k,m].T @ A[k,:]   (activations stay
    transposed, weights are the stationary operand)
  - layer 3 swaps operands: Z3[mb,n] = sum_k A[k,mb].T @ W2[k,n], which
    yields the output in natural [batch, channel] layout directly.
All matmuls in bf16 with fp32 PSUM accumulation. Weights and X are cast
fp32->bf16 inside the (software-DGE) DMA itself.

Loop order is k-OUTER with 8 concurrent PSUM accumulation chains per group
so the in-order PE consumes weight strips as they stream from HBM. Weight
strips are loaded as column halves (A: cols 0..1023, B: cols 1024..2047);
group 0 only needs the A halves, halving the ramp-up stall.
"""

import numpy as np

import concourse.bass as bass
import concourse.bacc as bacc
import concourse.mybir as mybir
import concourse.tile as tile
from concourse.bass_utils import run_bass_kernel_spmd
from concourse.masks import make_identity

B = 4096
C = 1024
NCORES = 8
BC = B // NCORES          # 512 rows per core
K = 2 * C                 # 2048 contraction / channel dim
P = 128                   # partitions
KT = K // P               # 16 k-tiles
MT = K // P               # 16 m-tiles (output channel tiles of 128)
NBAT = BC                 # 512, batch free size (fits one PSUM bank in f32)
G = 8                     # concurrent PSUM chains per group
KH = K // 2               # 1024, weight column half

F32 = mybir.dt.float32
BF16 = mybir.dt.bfloat16

_built = None


def _build():
    nc = bacc.Bacc(num_swdge_queues=4)
    x0 = nc.declare_dram_parameter("x0", [BC, C], F32, isOutput=False)
    x1 = nc.declare_dram_parameter("x1", [BC, C], F32, isOutput=False)
    ws = [
        nc.declare_dram_parameter(f"W{i}", [K, K], F32, isOutput=False)
        for i in range(3)
    ]
    out = nc.declare_dram_parameter("out", [BC, K], F32, isOutput=True)

    with tile.TileContext(nc) as tc:
        with (
            tc.tile_pool(name="xpool", bufs=8) as xpool,      # bf16 X row-tiles
            tc.tile_pool(name="wpool", bufs=2) as wpool,      # bf16 weight half-strips
            tc.tile_pool(name="act", bufs=2) as act,          # bf16 activation strips
            tc.tile_pool(name="outp", bufs=4) as outp,        # f32 out staging
            tc.tile_pool(name="psum", bufs=G, space=bass.MemorySpace.PSUM) as psum,
        ):
            # ---- X: casting DMA to bf16, then PE transpose into A0 ----
            ident = xpool.tile([P, P], BF16, tag="ident", name="ident")
            make_identity(nc, ident[:])

            a_in = []
            for k in range(KT):
                a_in.append(act.tile([P, NBAT], BF16, tag=f"a{k}", name=f"a_in{k}"))

            for r in range(BC // P):  # 4 row-tiles of X
                xh = []
                for h, src in enumerate((x0, x1)):
                    xb = xpool.tile([P, C], BF16, tag=f"xb{h}", name=f"xb{r}_{h}")
                    nc.gpsimd.dma_start(xb[:], src[r * P:(r + 1) * P, :])
                    xh.append(xb)
                for k in range(KT):
                    src = xh[k // (KT // 2)]
                    kk = k % (KT // 2)
                    pt = psum.tile([P, P], BF16, tag="ps", name=f"pt{r}_{k}")
                    nc.tensor.transpose(pt[:], src[:, kk * P:(kk + 1) * P], ident[:])
                    nc.vector.tensor_copy(a_in[k][:, r * P:(r + 1) * P], pt[:])

            # ---- weight loader: two column halves per k strip ----
            def load_w(layer):
                ha, hb = [], []
                for half, lst in ((0, ha), (1, hb)):
                    for k in range(KT):
                        wb = wpool.tile(
                            [P, KH], BF16,
                            tag=f"w{half}_{k}", name=f"w{layer}_{half}_{k}",
                        )
                        nc.gpsimd.dma_start(
                            wb[:],
                            ws[layer][k * P:(k + 1) * P, half * KH:(half + 1) * KH],
                        )
                        lst.append(wb)
                return ha, hb

            # ---- transposed-activation layer: k-outer, G chains per group ----
            def layer_t(whalves, ain, lname):
                nxt = []
                for gi, half in enumerate(whalves):  # group 0: cols 0..1023
                    chains = [
                        psum.tile([P, NBAT], F32, tag="ps",
                                  name=f"ps{lname}_{gi * G + j}")
                        for j in range(G)
                    ]
                    for k in range(KT):
                        for j in range(G):
                            nc.tensor.matmul(
                                chains[j][:],
                                half[k][:, j * P:(j + 1) * P],
                                ain[k][:],
                                start=(k == 0),
                                stop=(k == KT - 1),
                            )
                    for j in range(G):
                        m = gi * G + j
                        ao = act.tile(
                            [P, NBAT], BF16, tag=f"a{m}", name=f"a{lname}_{m}"
                        )
                        nc.vector.tensor_copy(ao[:], chains[j][:])
                        nxt.append(ao)
                return nxt

            w0a, w0b = load_w(0)
            a1 = layer_t((w0a, w0b), a_in, "L1")
            w1a, w1b = load_w(1)     # emitted after L1 so W0 wins DMA priority
            a2 = layer_t((w1a, w1b), a1, "L2")
            w2a, w2b = load_w(2)

            # ---- layer 3: natural-layout output, k-outer over (mb, n) pairs ----
            # n-major grouping: group 0 touches only the A halves of W2.
            pairs = [(mb, n) for n in range(K // NBAT) for mb in range(BC // P)]
            G3 = 4  # smaller groups: staggers the tail copies/stores
            for g0 in range(0, len(pairs), G3):
                grp = pairs[g0:g0 + G3]
                chains = [
                    psum.tile([P, NBAT], F32, tag="ps", name=f"psL3_{g0 + j}")
                    for j in range(len(grp))
                ]
                for k in range(KT):
                    for j, (mb, n) in enumerate(grp):
                        half = (w2a, w2b)[n // 2]
                        nc.tensor.matmul(
                            chains[j][:],
                            a2[k][:, mb * P:(mb + 1) * P],
                            half[k][:, (n % 2) * NBAT:(n % 2 + 1) * NBAT],
                            start=(k == 0),
                            stop=(k == KT - 1),
                        )
                for j, (mb, n) in enumerate(grp):
                    ob = outp.tile([P, NBAT], F32, tag="ob", name=f"ob{g0 + j}")
                    nc.vector.tensor_copy(ob[:], chains[j][:])
                    nc.sync.dma_start(
                        out[mb * P:(mb + 1) * P, n * NBAT:(n + 1) * NBAT], ob[:]
                    )

    nc.finalize()
    return nc


def _run(inputs, trace=False, **kw):
    global _built
    if _built is None:
        _built = _build()
    nc = _built
    in_maps = []
    for c in range(NCORES):
        sl = slice(c * BC, (c + 1) * BC)
        in_maps.append({
            "x0": np.ascontiguousarray(inputs["x0"][sl]),
            "x1": np.ascontiguousarray(inputs["x1"][sl]),
            "W0": inputs["W0"],
            "W1": inputs["W1"],
            "W2": inputs["W2"],
        })
    res = run_bass_kernel_spmd(nc, in_maps, list(range(NCORES)), trace=trace, **kw)
    out = np.concatenate([res.results[c]["out"] for c in range(NCORES)], axis=0)
    return out, res


def kernel(**inputs):
    out, _ = _run(inputs)
    return out
